# revision 1
# baseline (speedup 1.0000x reference)
"""GQA attention kernel for Trainium2, 8 NeuronCores.

Problem: B=2, T=2048, D=1024, 16 Q heads / 4 KV heads, head_dim=64, RoPE,
causal softmax, out-projection.

Sharding: 8 cores = 2 (batch) x 4 (KV group). Core c handles batch c//4 and
KV group g=c%4 (query heads 4g..4g+3). wq/wk/wv column-sharded, wo
row-sharded; the 4 partial outputs per batch are summed on the host.

On-chip layout: everything is kept transposed (head_dim on partitions):
  xT (D, T), qT (256, T), kT (64, T).  Scores are computed directly in
transposed orientation scoresT[j, i] = k_j . q_i (j on partitions), so no
on-chip transposes of the attention matrix are needed.  Softmax runs without
max-subtraction (scores are O(6) bounded), and the denominator L[i] is
obtained for free by augmenting V with a ones-column in the PV matmul.
RoPE pairs are de-interleaved via a host-side column permutation of wq/wk so
rotate-half applies; the interleave never needs to be undone because q and k
share the same permutation and V/out stay in natural order.

All matmuls run as float32r (full fp32 data, fast PE mode).  Engines have no
cross-partition paths, so every partition-base change (rotate-half swap, kT
duplication, odd-head placement) goes through SBUF->SBUF DMA.
"""

import numpy as np
import sys

sys.path.insert(0, "/opt/trn_rl_repo")

from concourse import bass, bacc, mybir, tile  # noqa: E402
from concourse.bass_utils import run_bass_kernel_spmd  # noqa: E402

F32 = mybir.dt.float32
F32R = mybir.dt.float32r

B, T, D = 2, 2048, 1024
HD = 64                      # head dim
NQH = 4                      # query heads per core
QCOLS = NQH * HD             # 256
KC = D // 128                # 8 contraction chunks
NT = T // 128                # 16 row tiles
NC4 = T // 512               # 4 512-wide column chunks
N_CORES = 8

_cache = {}


def _r(ap):
    return ap.bitcast(F32R)


def build_nc():
    """Build the (SPMD-identical) single-core bass program."""
    nc = bacc.Bacc("TRN2", target_bir_lowering=False, debug=False)

    xT_d = nc.declare_dram_parameter("xT", [D, T], F32R, isOutput=False)
    wq_d = nc.declare_dram_parameter("wq", [D, QCOLS], F32R, isOutput=False)
    wk_d = nc.declare_dram_parameter("wk", [D, HD], F32R, isOutput=False)
    wv_d = nc.declare_dram_parameter("wv", [D, HD], F32R, isOutput=False)
    wo_d = nc.declare_dram_parameter("wo", [QCOLS, D], F32R, isOutput=False)
    cos_d = nc.declare_dram_parameter("cosf", [128, T], F32, isOutput=False)
    sin_d = nc.declare_dram_parameter("sinf", [128, T], F32, isOutput=False)
    msk_d = nc.declare_dram_parameter("msk", [128, 4, 512], F32, isOutput=False)
    one_d = nc.declare_dram_parameter("onec", [128, HD], F32R, isOutput=False)
    out_d = nc.declare_dram_parameter("out", [T, D], F32, isOutput=True)

    with tile.TileContext(nc) as tc:
        with tc.tile_pool(name="sb", bufs=1) as sb:
            wq = sb.tile([128, KC, QCOLS], F32, tag="wq")
            wk = sb.tile([128, KC, HD], F32, tag="wk")
            wv = sb.tile([128, KC, HD], F32, tag="wv")
            wo = sb.tile([128, 2, D], F32, tag="wo")
            cosf = sb.tile([128, T], F32, tag="cosf")
            sinf = sb.tile([128, T], F32, tag="sinf")
            msk = sb.tile([128, 4, 512], F32, tag="msk")
            # ones row placed at partition 64 to align with the L row of the
            # PV accumulator (engines need matching partition bases).
            ones = sb.tile([65, HD], F32, tag="ones")
            qT = [sb.tile([128, T], F32, tag=f"qT{hp}", name=f"qT{hp}")
                  for hp in range(2)]
            # kT duplicated into both partition halves so scores matmuls can
            # read it at base partition 0 (even heads) or 64 (odd heads).
            kT = sb.tile([128, T], F32, tag="kT")
            v = sb.tile([128, NT, HD + 1], F32, tag="v")
            ao = [sb.tile([128, T], F32, tag=f"ao{hp}", name=f"ao{hp}")
                  for hp in range(2)]

            for k in range(KC):
                nc.sync.dma_start(_r(wq[:, k, :]), wq_d[k * 128:(k + 1) * 128, :])
                nc.sync.dma_start(_r(wk[:, k, :]), wk_d[k * 128:(k + 1) * 128, :])
                nc.sync.dma_start(_r(wv[:, k, :]), wv_d[k * 128:(k + 1) * 128, :])
            nc.sync.dma_start(cosf[:], cos_d[:])
            nc.sync.dma_start(sinf[:], sin_d[:])
            nc.sync.dma_start(msk[:], msk_d[:])
            for c in range(2):
                nc.sync.dma_start(_r(wo[:, c, :]), wo_d[c * 128:(c + 1) * 128, :])

            nc.sync.dma_start(_r(ones[64:65, :]), one_d[64:65, :])
            nc.sync.dma_start(_r(v[:, :, HD:HD + 1]), one_d[:, 0:NT])

            # --- projections (xT lives only here) ---
            with (
                tc.tile_pool(name="sbx", bufs=1) as sbx,
                tc.tile_pool(name="rope", bufs=1) as rope_pool,
                tc.tile_pool(name="ppsum", bufs=2, space="PSUM") as ppsum,
            ):
                xT = sbx.tile([128, KC, T], F32, tag="xT")
                for k in range(KC):
                    nc.sync.dma_start(_r(xT[:, k, :]), xT_d[k * 128:(k + 1) * 128, :])

                def rope_inplace(q_ap, nrows):
                    """q = q*cos + rot_half(q)*sin, on de-interleaved rows."""
                    rot = rope_pool.tile([128, T], F32, tag="rot")
                    for blk in range(nrows // 64):
                        r0 = blk * 64
                        nc.sync.dma_start(rot[r0:r0 + 32, :],
                                          q_ap[r0 + 32:r0 + 64, :])
                        nc.sync.dma_start(rot[r0 + 32:r0 + 64, :],
                                          q_ap[r0:r0 + 32, :])
                    nc.vector.tensor_mul(_r(q_ap[0:nrows, :]), q_ap[0:nrows, :],
                                         cosf[0:nrows, :])
                    nc.vector.tensor_mul(rot[0:nrows, :], rot[0:nrows, :],
                                         sinf[0:nrows, :])
                    nc.vector.tensor_add(_r(q_ap[0:nrows, :]), q_ap[0:nrows, :],
                                         rot[0:nrows, :])

                for hp in range(2):
                    pq = ppsum.tile([128, T], F32, tag="proj")
                    for ci in range(NC4):
                        cs = slice(ci * 512, (ci + 1) * 512)
                        for k in range(KC):
                            nc.tensor.matmul(
                                pq[:, cs],
                                _r(wq[:, k, hp * 128:(hp + 1) * 128]),
                                _r(xT[:, k, cs]),
                                start=(k == 0), stop=(k == KC - 1))
                    nc.scalar.copy(_r(qT[hp][:]), pq[:])
                    rope_inplace(qT[hp][:], 128)

                pk = ppsum.tile([64, T], F32, tag="proj")
                for ci in range(NC4):
                    cs = slice(ci * 512, (ci + 1) * 512)
                    for k in range(KC):
                        nc.tensor.matmul(
                            pk[:, cs], _r(wk[:, k, :]), _r(xT[:, k, cs]),
                            start=(k == 0), stop=(k == KC - 1))
                nc.scalar.copy(_r(kT[0:64, :]), pk[:])
                rope_inplace(kT[:], 64)
                nc.sync.dma_start(_r(kT[64:128, :]), _r(kT[0:64, :]))

                for t in range(NT):
                    pv = ppsum.tile([128, HD], F32, tag="proj")
                    for k in range(KC):
                        nc.tensor.matmul(
                            pv[:], _r(xT[:, k, t * 128:(t + 1) * 128]),
                            _r(wv[:, k, :]),
                            start=(k == 0), stop=(k == KC - 1))
                    nc.scalar.copy(_r(v[:, t, 0:HD]), pv[:])

            # --- attention, one KV head (4 query heads) ---
            with (
                tc.tile_pool(name="aox", bufs=2) as aox,
                tc.tile_pool(name="at", bufs=6) as at_pool,
                tc.tile_pool(name="pvpsum", bufs=1, space="PSUM") as pvp,
                tc.tile_pool(name="scpsum", bufs=3, space="PSUM") as scp,
            ):
                for h in range(NQH):
                    hp, hr = divmod(h, 2)
                    qrow = slice(hr * 64, hr * 64 + 64)
                    pv_acc = pvp.tile([HD + 1, T], F32, tag="pv")
                    for ci in range(NC4):
                        cs = slice(ci * 512, (ci + 1) * 512)
                        n_tj = (ci + 1) * 4
                        for tj in range(n_tj):
                            sc = scp.tile([128, 512], F32, tag="sc")
                            nc.tensor.matmul(
                                sc[:],
                                _r(kT[qrow, tj * 128:(tj + 1) * 128]),
                                _r(qT[hp][qrow, cs]),
                                start=True, stop=True)
                            if tj >= ci * 4:  # diagonal block: causal mask
                                nc.vector.tensor_add(
                                    sc[:], sc[:], msk[:, tj - ci * 4, :])
                            at = at_pool.tile([128, 512], F32, tag="at")
                            nc.scalar.activation(
                                _r(at[:]), sc[:],
                                mybir.ActivationFunctionType.Exp,
                                scale=0.125)
                            nc.tensor.matmul(
                                pv_acc[:, cs], _r(v[:, tj, :]), _r(at[:]),
                                start=(tj == 0), stop=(tj == n_tj - 1))
                    # normalize: ao rows of head h = pv_acc[0:64] * (1/L);
                    # L sits in pv_acc row 64 (the ones-column of v_aug).
                    linv = aox.tile([65, T], F32, tag="linv")
                    with nc.allow_low_precision(reason="fp32r linv"):
                        nc.vector.reciprocal(_r(linv[64:65, :]),
                                             pv_acc[HD:HD + 1, :])
                    if hr == 0:
                        dst = ao[hp][0:64, :]
                    else:
                        dst = aox.tile([64, T], F32, tag="aotmp")
                    nc.scalar.copy(_r(dst), pv_acc[0:HD, :])
                    for ci in range(NC4):
                        cs = slice(ci * 512, (ci + 1) * 512)
                        lb = scp.tile([HD, 512], F32, tag="sc")
                        nc.tensor.matmul(lb[:], _r(ones[64:65, :]),
                                         _r(linv[64:65, cs]),
                                         start=True, stop=True)
                        nc.vector.tensor_mul(_r(dst[:, cs]), dst[:, cs], lb[:])
                    if hr == 1:
                        nc.sync.dma_start(_r(ao[hp][64:128, :]), _r(dst))

            # --- output projection ---
            with (
                tc.tile_pool(name="outp", bufs=3) as outp,
                tc.tile_pool(name="wopsum", bufs=2, space="PSUM") as wop,
            ):
                for t in range(NT):
                    po = wop.tile([128, D], F32, tag="po")
                    for nh in range(2):
                        ns = slice(nh * 512, (nh + 1) * 512)
                        for cc in range(2):
                            nc.tensor.matmul(
                                po[:, ns],
                                _r(ao[cc][:, t * 128:(t + 1) * 128]),
                                _r(wo[:, cc, ns]),
                                start=(cc == 0), stop=(cc == 1))
                    ot = outp.tile([128, D], F32, tag="ot")
                    nc.scalar.copy(ot[:], po[:])
                    nc.sync.dma_start(out_d[t * 128:(t + 1) * 128, :], ot[:])

    nc.compile()
    return nc


def _round_f32r(a):
    """Round fp32 to the fp32r grid (11-bit mantissa, round-to-nearest)."""
    bits = np.ascontiguousarray(a, np.float32).view(np.uint32)
    return ((bits + 0x800) & 0xFFFFF000).view(np.float32)


def make_in_maps(x, freqs_cos, freqs_sin, wq, wk, wv, wo):
    """Host-side sharding + layout prep. Returns per-core input dicts."""
    x = np.asarray(x, np.float32)
    fc = np.asarray(freqs_cos, np.float32)
    fs = np.asarray(freqs_sin, np.float32)
    wq = np.asarray(wq, np.float32)
    wk = np.asarray(wk, np.float32)
    wv = np.asarray(wv, np.float32)
    wo = np.asarray(wo, np.float32)

    perm = np.concatenate([np.arange(0, HD, 2), np.arange(1, HD, 2)])
    cosT = np.ascontiguousarray(fc.T)            # (32, T)
    sinT = np.ascontiguousarray(fs.T)
    cosf = np.concatenate([cosT] * 4, axis=0)    # (128, T)
    sinf = np.concatenate([-sinT, sinT, -sinT, sinT], axis=0)

    jj = np.arange(128)[:, None]
    ii = np.arange(512)[None, :]
    msk = np.stack(
        [np.where(r * 128 + jj <= ii, 0.0, -1e30) for r in range(4)], axis=0
    ).astype(np.float32)                         # (4, 128, 512)
    mskT = np.ascontiguousarray(msk.transpose(1, 0, 2))  # (128, 4, 512)

    in_maps = []
    for c in range(N_CORES):
        b, g = divmod(c, 4)
        wq_c = wq[:, g * QCOLS:(g + 1) * QCOLS]
        wq_c = np.ascontiguousarray(
            wq_c.reshape(D, NQH, HD)[:, :, perm].reshape(D, QCOLS))
        wk_c = np.ascontiguousarray(wk[:, g * HD:(g + 1) * HD][:, perm])
        wv_c = np.ascontiguousarray(wv[:, g * HD:(g + 1) * HD])
        wo_c = np.ascontiguousarray(wo[g * QCOLS:(g + 1) * QCOLS, :])
        xT_c = np.ascontiguousarray(x[b].T)
        in_maps.append({
            "xT": _round_f32r(xT_c), "wq": _round_f32r(wq_c),
            "wk": _round_f32r(wk_c), "wv": _round_f32r(wv_c),
            "wo": _round_f32r(wo_c),
            "cosf": cosf, "sinf": sinf, "msk": mskT,
            "onec": np.ones((128, HD), np.float32),
        })
    return in_maps


def run_on_cores(in_maps, trace=False, **kwargs):
    if "nc" not in _cache:
        _cache["nc"] = build_nc()
    return run_bass_kernel_spmd(
        _cache["nc"], in_maps, core_ids=list(range(N_CORES)), trace=trace,
        **kwargs)


def kernel(x, freqs_cos, freqs_sin, wq, wk, wv, wo):
    in_maps = make_in_maps(x, freqs_cos, freqs_sin, wq, wk, wv, wo)
    res = run_on_cores(in_maps)
    outs = [res.results[c]["out"] for c in range(N_CORES)]
    full = np.empty((B, T, D), np.float32)
    for b in range(B):
        full[b] = outs[4 * b] + outs[4 * b + 1] + outs[4 * b + 2] + outs[4 * b + 3]
    return full



# revision 24
# speedup vs baseline: 1.0060x; 1.0060x over previous
"""GQA attention kernel for Trainium2, 8 NeuronCores.

Problem: B=2, T=2048, D=1024, 16 Q heads / 4 KV heads, head_dim=64, RoPE,
causal softmax, out-projection.

Sharding: 8 cores = 2 (batch) x 4 (KV group). Core c handles batch c//4 and
KV group g=c%4 (query heads 4g..4g+3). wq/wk/wv column-sharded, wo
row-sharded; the 4 partial outputs per batch are summed on the host.

v2 architecture (vs the v1 baseline at 428us):
- Pipelined over 4 column chunks of T (512 q-positions each): projections,
  attention, and the output projection of the previous chunk interleave, so
  the PE never drains and the HBM loads/stores overlap compute.
- Softmax exp is split between the Scalar engine (native Exp activation) and
  the Vector engine (Schraudolph bit-trick exp: i32 = s*A + B, bitcast to
  f32 gives 2^(s*log2e*0.125) with ~3% sawtooth error that mostly cancels in
  the softmax normalization). One exp instruction covers a PAIR of score
  tiles ([128, 2, 512] PSUM) to amortize per-instruction overhead.
- Scores are computed transposed (scoresT[kpos, qpos]) so PV needs no
  transposes; the softmax denominator L rides along as a ones-column of V.
- 1/L is computed on a [128, 4] tile (DMA reshape of the one-partition L row)
  instead of a [1, 512] row: the Vector reciprocal is per-lane-serial, so the
  reshape makes it ~100x cheaper.
- Causal trimming: diagonal score tiles only compute/exp/PV the valid
  column range; the triangular boundary block gets a [128,128] mask add.
- RoPE multiplies run on the (otherwise idle) GpSimd engine; V is produced
  via PE transposes of the K/V projection; all PSUM<->SBUF copies are on
  Scalar, masks/normalize/reciprocal on Vector.

All matmuls are float32r (full fp32 data, fast PE mode).
"""

import numpy as np
import sys

sys.path.insert(0, "/opt/trn_rl_repo")

from concourse import bass, bacc, mybir, tile  # noqa: E402
from concourse.bass_utils import run_bass_kernel_spmd  # noqa: E402

F32 = mybir.dt.float32
F32R = mybir.dt.float32r
I16 = mybir.dt.int16
BF16 = mybir.dt.bfloat16

B, T, D = 2, 2048, 1024
HD = 64                      # head dim
NQH = 4                      # query heads per core
QCOLS = NQH * HD             # 256
KC = D // 128                # 8 contraction chunks
NCI = 4                      # 512-wide column chunks of T
NT = T // 128                # 16 k-position tiles
N_CORES = 8

LOG2E = 1.4426950408889634
EXPA = 0.125 * LOG2E * (1 << 7)           # fold the 1/sqrt(hd) scale in
EXPB = (127.0 - 0.05) * (1 << 7)          # Schraudolph bias, tuned offline
MASKV = -300.0

_cache = {}


def _r(ap):
    return ap.bitcast(F32R)


def build_nc():
    """Build the (SPMD-identical) single-core bass program."""
    nc = bacc.Bacc("TRN2", target_bir_lowering=False, debug=False)

    xT_d = nc.declare_dram_parameter("xT", [D, T], F32R, isOutput=False)
    wq_d = nc.declare_dram_parameter("wq", [D, QCOLS], F32R, isOutput=False)
    wkv_d = nc.declare_dram_parameter("wkv", [D, 128], F32R, isOutput=False)
    wo_d = nc.declare_dram_parameter("wo", [QCOLS, D], F32R, isOutput=False)
    cos_d = nc.declare_dram_parameter("cosf", [128, T], F32, isOutput=False)
    sin_d = nc.declare_dram_parameter("sinf", [128, T], F32, isOutput=False)
    msk_d = nc.declare_dram_parameter("msk", [128, 128], F32, isOutput=False)
    id_d = nc.declare_dram_parameter("ident", [128, HD], F32, isOutput=False)
    out_d = nc.declare_dram_parameter("out", [T, D], F32, isOutput=True)
    # DRAM scratch used to reshape 1/L rows ([128,4] -> [1,512]); the DMA
    # engine is the only path that can move data across SBUF partitions.
    scr_d = nc.declare_dram_parameter("scr", [16, 512], F32R, isOutput=True)
    ones_d = nc.declare_dram_parameter("ones1", [1, HD], F32R, isOutput=False)

    with tile.TileContext(nc) as tc:
        with (
            tc.tile_pool(name="sb", bufs=1) as sb,
            tc.tile_pool(name="atp", bufs=3) as atp,
            tc.tile_pool(name="aop", bufs=4) as aop,
            tc.tile_pool(name="aotp", bufs=2) as aotp,
            tc.tile_pool(name="otp", bufs=3) as otp,
            tc.tile_pool(name="rotp", bufs=2) as rotp,
            tc.tile_pool(name="vtp", bufs=2) as vtp,
            tc.tile_pool(name="lrp", bufs=2) as lrp,
            tc.tile_pool(name="scp", bufs=2, space="PSUM") as scp,
            tc.tile_pool(name="pvp", bufs=2, space="PSUM") as pvp,
            tc.tile_pool(name="pop", bufs=2, space="PSUM") as pop,
        ):
            wq = sb.tile([128, KC, QCOLS], F32, tag="wq")
            wkv = sb.tile([128, KC, 128], F32, tag="wkv")
            wo = sb.tile([128, 2, D], F32, tag="wo")
            cosf = sb.tile([128, T], F32, tag="cosf")
            sinf = sb.tile([128, T], F32, tag="sinf")
            msk = sb.tile([128, 128], F32, tag="msk")
            ident = sb.tile([128, HD], F32, tag="ident")
            qT = [sb.tile([128, T], F32, tag=f"qT{hp}", name=f"qT{hp}")
                  for hp in range(2)]
            kT = sb.tile([128, T], F32, tag="kT")
            vs = sb.tile([128, NT, HD + 1], BF16, tag="vs")
            xc = [sb.tile([128, KC, 512], F32, tag=f"xc{ci}", name=f"xc{ci}")
                  for ci in range(NCI)]

            # --- input loads (order = DMA issue order on the sync queue) ---
            nc.sync.dma_start(
                _r(wq[:]), wq_d[:, :].rearrange("(a b) c -> b a c", a=KC))
            nc.sync.dma_start(
                _r(wkv[:]), wkv_d[:, :].rearrange("(a b) c -> b a c", a=KC))
            nc.sync.dma_start(
                _r(xc[0][:]),
                xT_d[:, 0:512].rearrange("(a b) c -> b a c", a=KC))
            nc.sync.dma_start(cosf[:], cos_d[:])
            nc.sync.dma_start(sinf[:], sin_d[:])
            nc.sync.dma_start(msk[:], msk_d[:])
            nc.sync.dma_start(ident[:], id_d[:])
            nc.sync.dma_start(
                _r(xc[1][:]),
                xT_d[:, 512:1024].rearrange("(a b) c -> b a c", a=KC))
            nc.sync.dma_start(
                _r(wo[:]), wo_d[:, :].rearrange("(a b) c -> b a c", a=2))
            for ci in range(2, NCI):
                nc.sync.dma_start(
                    _r(xc[ci][:]),
                    xT_d[:, ci * 512:(ci + 1) * 512].rearrange(
                        "(a b) c -> b a c", a=KC))

            ones1 = sb.tile([1, HD], F32, tag="ones1")
            nc.sync.dma_start(_r(ones1[:]), ones_d[:])
            nc.vector.memset(vs[:, :, HD:HD + 1], 1.0)

            def rope_chunk(t_ap, cs, nrows):
                """t = t*cos + rot_half(t)*sin on de-interleaved rows.

                rot DMAs swap 32-row halves of each 64 block; muls/adds run
                on GpSimd to keep Vector/Scalar free for softmax work.
                """
                rot = rotp.tile([128, 512], F32, tag="rot")
                for blk in range(nrows // 64):
                    r0 = blk * 64
                    nc.scalar.dma_start(rot[r0:r0 + 32, :],
                                        t_ap[r0 + 32:r0 + 64, cs])
                    nc.scalar.dma_start(rot[r0 + 32:r0 + 64, :],
                                        t_ap[r0:r0 + 32, cs])
                nc.gpsimd.tensor_mul(_r(t_ap[0:nrows, cs]), t_ap[0:nrows, cs],
                                     cosf[0:nrows, cs])
                nc.gpsimd.tensor_mul(rot[0:nrows, :], rot[0:nrows, :],
                                     sinf[0:nrows, cs])
                nc.gpsimd.tensor_add(_r(t_ap[0:nrows, cs]), t_ap[0:nrows, cs],
                                     rot[0:nrows, :])

            def proj(ci):
                cs = slice(ci * 512, (ci + 1) * 512)
                for hp in range(2):
                    pq = pop.tile([128, 512], F32, tag="po")
                    for k in range(KC):
                        nc.tensor.matmul(
                            pq[:], _r(wq[:, k, hp * 128:(hp + 1) * 128]),
                            _r(xc[ci][:, k, :]),
                            start=(k == 0), stop=(k == KC - 1))
                    nc.scalar.copy(_r(qT[hp][:, cs]), pq[:])
                    rope_chunk(qT[hp], cs, 128)
                pkv = pop.tile([128, 512], F32, tag="po")
                for k in range(KC):
                    nc.tensor.matmul(
                        pkv[:], _r(wkv[:, k, :]), _r(xc[ci][:, k, :]),
                        start=(k == 0), stop=(k == KC - 1))
                nc.scalar.copy(_r(kT[0:64, cs]), pkv[0:64, :])
                vtmp = vtp.tile([128, 512], F32, tag="vtmp")
                nc.scalar.copy(vtmp[64:128, :], pkv[64:128, :])
                rope_chunk(kT, cs, 64)
                # duplicate roped kT into the upper partition half for the
                # odd-head score matmuls (engines can't cross partitions).
                nc.scalar.dma_start(_r(kT[64:128, cs]), _r(kT[0:64, cs]))
                # transpose V chunks into natural [tpos, dim] orientation
                for tb in range(4):
                    pt = pop.tile([128, 512], F32, tag="po")
                    nc.tensor.transpose(
                        pt[:, 0:HD], vtmp[64:128, tb * 128:(tb + 1) * 128],
                        ident[64:128, 0:HD])
                    nc.scalar.copy(vs[:, ci * 4 + tb, 0:HD], pt[:, 0:HD])

            def attn_head(h, ci, state):
                cs0 = ci * 512
                n_tj = 4 * (ci + 1)
                hp, hr = divmod(h, 2)
                qrow = slice(hr * 64, hr * 64 + 64)
                pv = pvp.tile([HD + 1, 512], F32, tag="pv")
                for p in range(n_tj // 2):
                    sc = scp.tile([128, 2, 512], F32, tag="sc")
                    at = atp.tile([128, 2, 512], BF16, tag="at")
                    m0 = 2 * p - 4 * ci
                    wp = 128 * m0 if m0 > 0 else 0
                    for s in range(2):
                        tj = 2 * p + s
                        nc.tensor.matmul(
                            sc[:, s, wp:512],
                            _r(kT[qrow, tj * 128:(tj + 1) * 128]),
                            _r(qT[hp][qrow, cs0 + wp:cs0 + 512]),
                            start=True, stop=True)
                    if p % 2 == 0:
                        nc.scalar.activation(
                            at[:, :, wp:512], sc[:, :, wp:512],
                            mybir.ActivationFunctionType.Exp, scale=0.125)
                    else:
                        nc.vector.tensor_scalar(
                            at[:, :, wp:512].bitcast(I16), sc[:, :, wp:512],
                            EXPA, EXPB, mybir.AluOpType.mult,
                            mybir.AluOpType.add)
                    for s in range(2):
                        # causal boundary: zero above-diagonal entries of the
                        # triangular block (0/1 mask; at is SBUF so the
                        # otherwise-idle GpSimd engine can do it).
                        tj = 2 * p + s
                        m = tj - 4 * ci
                        if m >= 0:
                            nc.gpsimd.tensor_mul(
                                at[:, s, 128 * m:128 * m + 128],
                                at[:, s, 128 * m:128 * m + 128], msk[:])
                    for s in range(2):
                        tj = 2 * p + s
                        m = tj - 4 * ci
                        w0 = 128 * m if m > 0 else 0
                        nc.tensor.matmul(
                            pv[:, w0:512], vs[:, tj, :],
                            at[:, s, w0:512],
                            start=(tj == 0), stop=(tj == n_tj - 1),
                            skip_group_check=True)
                state[h] = (pv, hp, hr)

            def attn_fin(h, ci, state, ao_ci):
                # ao = pv * (1/L). The L row sits on one PSUM partition where
                # every engine op is lane-serial, so: transpose 128-col chunks
                # onto partitions with the PE ([128,4]), reciprocal there,
                # DMA-bounce through DRAM back to a [1,512] row, broadcast it
                # over 64 partitions with a ones-stationary matmul, multiply.
                pv, hp, hr = state[h]
                idx = 4 * ci + h
                lr = lrp.tile([HD + 1, 512], F32, tag="lr")
                nc.scalar.copy(lr[HD:HD + 1, :], pv[HD:HD + 1, :])
                if hr == 0:
                    dst = ao_ci[hp][0:64, :]
                else:
                    tmp = aotp.tile([64, 512], F32, tag="aotmp", name="aotmp")
                    dst = tmp[:]
                nc.scalar.copy(_r(dst), pv[0:HD, :])  # frees pv for next head
                lcol = pop.tile([128, 512], F32, tag="po", name="lcol")
                for c in range(4):
                    nc.tensor.transpose(
                        lcol[:, c:c + 1],
                        lr[HD:HD + 1, c * 128:(c + 1) * 128],
                        ident[HD:HD + 1, 0:1])
                linvr = lrp.tile([128, 4], F32, tag="linvr")
                with nc.allow_low_precision(reason="f32r 1/L"):
                    nc.vector.reciprocal(_r(linvr[:]), lcol[:, 0:4])
                # linvr[p, c] = 1/L[128c + p]: scatter back in qpos order
                nc.sync.dma_start(
                    scr_d[idx:idx + 1, :].rearrange(
                        "a (f p) -> (a p) f", p=128), _r(linvr[:]))
                linv = lrp.tile([1, 512], F32, tag="linv")
                nc.sync.dma_start(_r(linv[:]), scr_d[idx:idx + 1, :])
                lb = pop.tile([128, 512], F32, tag="po", name="lb")
                nc.tensor.matmul(lb[0:64, :], _r(ones1[:]), _r(linv[:]),
                                 start=True, stop=True)
                nc.vector.tensor_mul(_r(dst), dst, lb[0:64, :])
                if hr == 1:
                    nc.scalar.dma_start(_r(ao_ci[hp][64:128, :]), _r(dst))

            def attn(ci, ao_ci):
                state = {}
                for h in range(NQH):
                    attn_head(h, ci, state)
                    attn_fin(h, ci, state, ao_ci)

            def outproj(ci, ao_ci):
                for tb in range(4):
                    ta = (ci * 4 + tb) * 128
                    ot = otp.tile([128, 2, 512], F32, tag="ot")
                    for nh in range(2):
                        po = pop.tile([128, 512], F32, tag="po")
                        for cc in range(2):
                            nc.tensor.matmul(
                                po[:],
                                _r(ao_ci[cc][:, tb * 128:(tb + 1) * 128]),
                                _r(wo[:, cc, nh * 512:(nh + 1) * 512]),
                                start=(cc == 0), stop=(cc == 1))
                        if nh == 0:
                            nc.scalar.copy(ot[:, nh, :], po[:])
                        else:
                            nc.vector.tensor_copy(ot[:, nh, :], po[:])
                    nc.sync.dma_start(out_d[ta:ta + 128, :], ot[:])

            ao_tiles = {}
            for ci in range(NCI):
                ao_tiles[ci] = [
                    aop.tile([128, 512], F32, tag="ao", name=f"ao{ci}_{hp}")
                    for hp in range(2)]
                proj(ci)
                if ci >= 1:
                    outproj(ci - 1, ao_tiles[ci - 1])
                attn(ci, ao_tiles[ci])
            outproj(NCI - 1, ao_tiles[NCI - 1])

    nc.compile()
    return nc


def _round_f32r(a):
    """Round fp32 to the fp32r grid (11-bit mantissa, round-to-nearest)."""
    bits = np.ascontiguousarray(a, np.float32).view(np.uint32)
    return ((bits + 0x800) & 0xFFFFF000).view(np.float32)


def make_in_maps(x, freqs_cos, freqs_sin, wq, wk, wv, wo):
    """Host-side sharding + layout prep. Returns per-core input dicts."""
    x = np.asarray(x, np.float32)
    fc = np.asarray(freqs_cos, np.float32)
    fs = np.asarray(freqs_sin, np.float32)
    wq = np.asarray(wq, np.float32)
    wk = np.asarray(wk, np.float32)
    wv = np.asarray(wv, np.float32)
    wo = np.asarray(wo, np.float32)

    perm = np.concatenate([np.arange(0, HD, 2), np.arange(1, HD, 2)])
    cosT = np.ascontiguousarray(fc.T)            # (32, T)
    sinT = np.ascontiguousarray(fs.T)
    cosf = np.concatenate([cosT] * 4, axis=0)    # (128, T)
    sinf = np.concatenate([-sinT, sinT, -sinT, sinT], axis=0)

    jj = np.arange(128)[:, None]
    ii = np.arange(128)[None, :]
    msk = np.where(jj <= ii, 1.0, 0.0).astype(np.float32)
    ident = np.tile(np.eye(HD, dtype=np.float32), (2, 1))

    in_maps = []
    for c in range(N_CORES):
        b, g = divmod(c, 4)
        wq_c = wq[:, g * QCOLS:(g + 1) * QCOLS]
        wq_c = np.ascontiguousarray(
            wq_c.reshape(D, NQH, HD)[:, :, perm].reshape(D, QCOLS))
        wk_c = wk[:, g * HD:(g + 1) * HD][:, perm]
        wv_c = wv[:, g * HD:(g + 1) * HD]
        wkv_c = np.ascontiguousarray(np.concatenate([wk_c, wv_c], axis=1))
        wo_c = np.ascontiguousarray(wo[g * QCOLS:(g + 1) * QCOLS, :])
        xT_c = np.ascontiguousarray(x[b].T)
        in_maps.append({
            "xT": _round_f32r(xT_c), "wq": _round_f32r(wq_c),
            "wkv": _round_f32r(wkv_c), "wo": _round_f32r(wo_c),
            "cosf": cosf, "sinf": sinf, "msk": msk, "ident": ident,
            "ones1": np.ones((1, HD), np.float32),
        })
    return in_maps


def run_on_cores(in_maps, trace=False, **kwargs):
    if "nc" not in _cache:
        _cache["nc"] = build_nc()
    return run_bass_kernel_spmd(
        _cache["nc"], in_maps, core_ids=list(range(N_CORES)), trace=trace,
        **kwargs)


def kernel(x, freqs_cos, freqs_sin, wq, wk, wv, wo):
    in_maps = make_in_maps(x, freqs_cos, freqs_sin, wq, wk, wv, wo)
    res = run_on_cores(in_maps)
    outs = [res.results[c]["out"] for c in range(N_CORES)]
    full = np.empty((B, T, D), np.float32)
    for b in range(B):
        full[b] = outs[4 * b] + outs[4 * b + 1] + outs[4 * b + 2] + outs[4 * b + 3]
    return full


# revision 28
# speedup vs baseline: 1.2928x; 1.2850x over previous
"""GQA attention kernel for Trainium2, 8 NeuronCores.

Problem: B=2, T=2048, D=1024, 16 Q heads / 4 KV heads, head_dim=64, RoPE,
causal softmax, out-projection.

Sharding: 8 cores = 2 (batch) x 4 (KV group). Core c handles batch c//4 and
KV group g=c%4 (query heads 4g..4g+3). wq/wk/wv column-sharded, wo
row-sharded; the 4 partial outputs per batch are summed on the host.

v2 architecture (vs the v1 baseline at 428us):
- Pipelined over 4 column chunks of T (512 q-positions each): projections,
  attention, and the output projection of the previous chunk interleave, so
  the PE never drains and the HBM loads/stores overlap compute.
- Softmax exp is split between the Scalar engine (native Exp activation) and
  the Vector engine (Schraudolph bit-trick exp: i32 = s*A + B, bitcast to
  f32 gives 2^(s*log2e*0.125) with ~3% sawtooth error that mostly cancels in
  the softmax normalization). One exp instruction covers a PAIR of score
  tiles ([128, 2, 512] PSUM) to amortize per-instruction overhead.
- Scores are computed transposed (scoresT[kpos, qpos]) so PV needs no
  transposes; the softmax denominator L rides along as a ones-column of V.
- 1/L is computed on a [128, 4] tile (DMA reshape of the one-partition L row)
  instead of a [1, 512] row: the Vector reciprocal is per-lane-serial, so the
  reshape makes it ~100x cheaper.
- Causal trimming: diagonal score tiles only compute/exp/PV the valid
  column range; the triangular boundary block gets a [128,128] mask add.
- RoPE multiplies run on the (otherwise idle) GpSimd engine; V is produced
  via PE transposes of the K/V projection; all PSUM<->SBUF copies are on
  Scalar, masks/normalize/reciprocal on Vector.

All matmuls are float32r (full fp32 data, fast PE mode).
"""

import numpy as np
import sys

sys.path.insert(0, "/opt/trn_rl_repo")

from concourse import bass, bacc, mybir, tile  # noqa: E402
from concourse.bass_utils import run_bass_kernel_spmd  # noqa: E402

F32 = mybir.dt.float32
F32R = mybir.dt.float32r
I16 = mybir.dt.int16
BF16 = mybir.dt.bfloat16

B, T, D = 2, 2048, 1024
HD = 64                      # head dim
NQH = 4                      # query heads per core
QCOLS = NQH * HD             # 256
KC = D // 128                # 8 contraction chunks
NCI = 4                      # 512-wide column chunks of T
NT = T // 128                # 16 k-position tiles
N_CORES = 8

LOG2E = 1.4426950408889634
EXPA = 0.125 * LOG2E * (1 << 7)           # fold the 1/sqrt(hd) scale in
EXPB = (127.0 - 0.05) * (1 << 7)          # Schraudolph bias, tuned offline
MASKV = -300.0

_cache = {}


def _r(ap):
    return ap.bitcast(F32R)


def build_nc():
    """Build the (SPMD-identical) single-core bass program."""
    nc = bacc.Bacc("TRN2", target_bir_lowering=False, debug=False)

    xT_d = nc.declare_dram_parameter("xT", [D, T], F32R, isOutput=False)
    wq_d = nc.declare_dram_parameter("wq", [D, QCOLS], F32R, isOutput=False)
    wkv_d = nc.declare_dram_parameter("wkv", [D, 128], F32R, isOutput=False)
    wo_d = nc.declare_dram_parameter("wo", [QCOLS, D], F32R, isOutput=False)
    cos_d = nc.declare_dram_parameter("cosf", [128, T], F32, isOutput=False)
    sin_d = nc.declare_dram_parameter("sinf", [128, T], F32, isOutput=False)
    msk_d = nc.declare_dram_parameter("msk", [128, 128], F32, isOutput=False)
    id_d = nc.declare_dram_parameter("ident", [128, HD], F32, isOutput=False)
    out_d = nc.declare_dram_parameter("out", [T, D], F32, isOutput=True)
    # DRAM scratch used to reshape 1/L rows ([128,4] -> [1,512]); the DMA
    # engine is the only path that can move data across SBUF partitions.
    scr_d = nc.declare_dram_parameter("scr", [16, 512], F32R, isOutput=True)
    ones_d = nc.declare_dram_parameter("ones1", [1, HD], F32R, isOutput=False)

    with tile.TileContext(nc) as tc:
        with (
            tc.tile_pool(name="sb", bufs=1) as sb,
            tc.tile_pool(name="atp", bufs=3) as atp,
            tc.tile_pool(name="aop", bufs=4) as aop,
            tc.tile_pool(name="aotp", bufs=2) as aotp,
            tc.tile_pool(name="otp", bufs=3) as otp,
            tc.tile_pool(name="rotp", bufs=2) as rotp,
            tc.tile_pool(name="vtp", bufs=2) as vtp,
            tc.tile_pool(name="lrp", bufs=2) as lrp,
            tc.tile_pool(name="scp", bufs=2, space="PSUM") as scp,
            tc.tile_pool(name="pvp", bufs=2, space="PSUM") as pvp,
            tc.tile_pool(name="pop", bufs=2, space="PSUM") as pop,
        ):
            wq = sb.tile([128, KC, QCOLS], F32, tag="wq")
            wkv = sb.tile([128, KC, 128], F32, tag="wkv")
            wo = sb.tile([128, 2, D], F32, tag="wo")
            cosf = sb.tile([128, T], F32, tag="cosf")
            sinf = sb.tile([128, T], F32, tag="sinf")
            msk = sb.tile([128, 128], F32, tag="msk")
            ident = sb.tile([128, HD], F32, tag="ident")
            qT = [sb.tile([128, T], F32, tag=f"qT{hp}", name=f"qT{hp}")
                  for hp in range(2)]
            kT = sb.tile([128, T], F32, tag="kT")
            vs = sb.tile([128, NT, HD + 1], BF16, tag="vs")
            xc = [sb.tile([128, KC, 512], F32, tag=f"xc{ci}", name=f"xc{ci}")
                  for ci in range(NCI)]

            # --- input loads (order = DMA issue order on the sync queue) ---
            nc.sync.dma_start(
                _r(wq[:]), wq_d[:, :].rearrange("(a b) c -> b a c", a=KC))
            nc.sync.dma_start(
                _r(wkv[:]), wkv_d[:, :].rearrange("(a b) c -> b a c", a=KC))
            nc.sync.dma_start(
                _r(xc[0][:]),
                xT_d[:, 0:512].rearrange("(a b) c -> b a c", a=KC))
            nc.sync.dma_start(cosf[:], cos_d[:])
            nc.sync.dma_start(sinf[:], sin_d[:])
            nc.sync.dma_start(msk[:], msk_d[:])
            nc.sync.dma_start(ident[:], id_d[:])
            nc.sync.dma_start(
                _r(xc[1][:]),
                xT_d[:, 512:1024].rearrange("(a b) c -> b a c", a=KC))
            nc.sync.dma_start(
                _r(wo[:]), wo_d[:, :].rearrange("(a b) c -> b a c", a=2))
            for ci in range(2, NCI):
                nc.sync.dma_start(
                    _r(xc[ci][:]),
                    xT_d[:, ci * 512:(ci + 1) * 512].rearrange(
                        "(a b) c -> b a c", a=KC))

            ones1 = sb.tile([1, HD], F32, tag="ones1")
            nc.sync.dma_start(_r(ones1[:]), ones_d[:])
            nc.vector.memset(vs[:, :, HD:HD + 1], 1.0)

            def rope_chunk(t_ap, cs, nrows):
                """t = t*cos + rot_half(t)*sin on de-interleaved rows.

                rot DMAs swap 32-row halves of each 64 block; muls/adds run
                on GpSimd to keep Vector/Scalar free for softmax work.
                """
                rot = rotp.tile([128, 512], F32, tag="rot")
                for blk in range(nrows // 64):
                    r0 = blk * 64
                    nc.scalar.dma_start(rot[r0:r0 + 32, :],
                                        t_ap[r0 + 32:r0 + 64, cs])
                    nc.scalar.dma_start(rot[r0 + 32:r0 + 64, :],
                                        t_ap[r0:r0 + 32, cs])
                nc.gpsimd.tensor_mul(_r(t_ap[0:nrows, cs]), t_ap[0:nrows, cs],
                                     cosf[0:nrows, cs])
                nc.gpsimd.tensor_mul(rot[0:nrows, :], rot[0:nrows, :],
                                     sinf[0:nrows, cs])
                nc.gpsimd.tensor_add(_r(t_ap[0:nrows, cs]), t_ap[0:nrows, cs],
                                     rot[0:nrows, :])

            def proj(ci):
                cs = slice(ci * 512, (ci + 1) * 512)
                for hp in range(2):
                    pq = pop.tile([128, 512], F32, tag="po")
                    for k in range(KC):
                        nc.tensor.matmul(
                            pq[:], _r(wq[:, k, hp * 128:(hp + 1) * 128]),
                            _r(xc[ci][:, k, :]),
                            start=(k == 0), stop=(k == KC - 1))
                    nc.scalar.copy(_r(qT[hp][:, cs]), pq[:])
                    rope_chunk(qT[hp], cs, 128)
                pkv = pop.tile([128, 512], F32, tag="po")
                for k in range(KC):
                    nc.tensor.matmul(
                        pkv[:], _r(wkv[:, k, :]), _r(xc[ci][:, k, :]),
                        start=(k == 0), stop=(k == KC - 1))
                nc.scalar.copy(_r(kT[0:64, cs]), pkv[0:64, :])
                vtmp = vtp.tile([128, 512], F32, tag="vtmp")
                nc.scalar.copy(vtmp[64:128, :], pkv[64:128, :])
                rope_chunk(kT, cs, 64)
                # duplicate roped kT into the upper partition half for the
                # odd-head score matmuls (engines can't cross partitions).
                nc.sync.dma_start(_r(kT[64:128, cs]), _r(kT[0:64, cs]))
                # transpose V chunks into natural [tpos, dim] orientation
                for tb in range(4):
                    pt = pop.tile([128, 512], F32, tag="po")
                    nc.tensor.transpose(
                        pt[:, 0:HD], vtmp[64:128, tb * 128:(tb + 1) * 128],
                        ident[64:128, 0:HD])
                    nc.scalar.copy(vs[:, ci * 4 + tb, 0:HD], pt[:, 0:HD])

            def attn_head(h, ci, state):
                cs0 = ci * 512
                n_tj = 4 * (ci + 1)
                hp, hr = divmod(h, 2)
                qrow = slice(hr * 64, hr * 64 + 64)
                pv = pvp.tile([HD + 1, 512], F32, tag="pv")
                for p in range(n_tj // 2):
                    sc = scp.tile([128, 2, 512], F32, tag="sc")
                    at = atp.tile([128, 2, 512], BF16, tag="at")
                    m0 = 2 * p - 4 * ci
                    wp = 128 * m0 if m0 > 0 else 0
                    for s in range(2):
                        tj = 2 * p + s
                        nc.tensor.matmul(
                            sc[:, s, wp:512],
                            _r(kT[qrow, tj * 128:(tj + 1) * 128]),
                            _r(qT[hp][qrow, cs0 + wp:cs0 + 512]),
                            start=True, stop=True)
                    if p % 2 == 0:
                        nc.scalar.activation(
                            at[:, :, wp:512], sc[:, :, wp:512],
                            mybir.ActivationFunctionType.Exp, scale=0.125)
                    else:
                        nc.vector.tensor_scalar(
                            at[:, :, wp:512].bitcast(I16), sc[:, :, wp:512],
                            EXPA, EXPB, mybir.AluOpType.mult,
                            mybir.AluOpType.add)
                    for s in range(2):
                        # causal boundary: zero above-diagonal entries of the
                        # triangular block (0/1 mask; at is SBUF so the
                        # otherwise-idle GpSimd engine can do it).
                        tj = 2 * p + s
                        m = tj - 4 * ci
                        if m >= 0:
                            nc.gpsimd.tensor_mul(
                                at[:, s, 128 * m:128 * m + 128],
                                at[:, s, 128 * m:128 * m + 128], msk[:])
                    for s in range(2):
                        tj = 2 * p + s
                        m = tj - 4 * ci
                        w0 = 128 * m if m > 0 else 0
                        nc.tensor.matmul(
                            pv[:, w0:512], vs[:, tj, :],
                            at[:, s, w0:512],
                            start=(tj == 0), stop=(tj == n_tj - 1),
                            skip_group_check=True)
                state[h] = (pv, hp, hr, None, None)

            def fin_a(h, ci, state):
                # Drain pv: copy L row + unnormalized ao to SBUF, transpose
                # the L row onto partitions ([128,4]) and take 1/L there
                # (engine ops are lane-serial on a 1-partition row, and DMA
                # is the only path that crosses partitions).
                pv, hp, hr, _, _ = state[h]
                lr = lrp.tile([HD + 1, 512], F32, tag="lr")
                nc.scalar.copy(lr[HD:HD + 1, :], pv[HD:HD + 1, :])
                if hr == 0:
                    dst = ao_tiles[ci][hp][0:64, :]
                else:
                    tmp = aotp.tile([64, 512], F32, tag="aotmp", name="aotmp")
                    dst = tmp[:]
                nc.scalar.copy(_r(dst), pv[0:HD, :])  # frees pv for next head
                lcol = pop.tile([128, 512], F32, tag="po", name="lcol")
                for c in range(4):
                    nc.tensor.transpose(
                        lcol[:, c:c + 1],
                        lr[HD:HD + 1, c * 128:(c + 1) * 128],
                        ident[HD:HD + 1, 0:1])
                linvr = lrp.tile([128, 4], F32, tag="linvr")
                with nc.allow_low_precision(reason="f32r 1/L"):
                    nc.vector.reciprocal(_r(linvr[:]), lcol[:, 0:4])
                state[h] = (pv, hp, hr, dst, linvr)

            def fin_b1(h, ci, state):
                # linvr[p, c] = 1/L[128c + p]: scatter back into qpos order.
                # Emitted one head late so the sync queue never waits on it.
                _, _, _, _, linvr = state[h]
                idx = 4 * ci + h
                nc.sync.dma_start(
                    scr_d[idx:idx + 1, :].rearrange(
                        "a (f p) -> (a p) f", p=128), _r(linvr[:]))

            def fin_b2(h, ci, state):
                # Gather 1/L as a [1,512] row, broadcast across 64 partitions
                # with a ones-stationary matmul, normalize ao in place.
                _, hp, hr, dst, _ = state[h]
                idx = 4 * ci + h
                linv = lrp.tile([1, 512], F32, tag="linv")
                nc.sync.dma_start(_r(linv[:]), scr_d[idx:idx + 1, :])
                lb = pop.tile([128, 512], F32, tag="po", name="lb")
                nc.tensor.matmul(lb[0:64, :], _r(ones1[:]), _r(linv[:]),
                                 start=True, stop=True)
                nc.vector.tensor_mul(_r(dst), dst, lb[0:64, :])
                if hr == 1:
                    nc.sync.dma_start(_r(ao_tiles[ci][hp][64:128, :]),
                                      _r(dst))

            def attn(ci, ao_ci):
                state = {}
                for h in range(NQH):
                    attn_head(h, ci, state)
                    fin_a(h, ci, state)
                    if h >= 1:
                        fin_b1(h - 1, ci, state)
                    if h >= 2:
                        fin_b2(h - 2, ci, state)
                fin_b1(NQH - 1, ci, state)
                fin_b2(NQH - 2, ci, state)
                fin_b2(NQH - 1, ci, state)

            def outproj(ci, ao_ci):
                for tb in range(4):
                    ta = (ci * 4 + tb) * 128
                    ot = otp.tile([128, 2, 512], F32, tag="ot")
                    for nh in range(2):
                        po = pop.tile([128, 512], F32, tag="po")
                        for cc in range(2):
                            nc.tensor.matmul(
                                po[:],
                                _r(ao_ci[cc][:, tb * 128:(tb + 1) * 128]),
                                _r(wo[:, cc, nh * 512:(nh + 1) * 512]),
                                start=(cc == 0), stop=(cc == 1))
                        if nh == 0:
                            nc.scalar.copy(ot[:, nh, :], po[:])
                        else:
                            nc.vector.tensor_copy(ot[:, nh, :], po[:])
                    nc.sync.dma_start(out_d[ta:ta + 128, :], ot[:])

            ao_tiles = {}
            for ci in range(NCI):
                ao_tiles[ci] = [
                    aop.tile([128, 512], F32, tag="ao", name=f"ao{ci}_{hp}")
                    for hp in range(2)]
            # emission order: run projections two chunks ahead of attention
            # so Pool/DMA work for chunk ci+2 fills attention-ci stalls, and
            # out-projection of ci lands right after proj(ci+2) on the PE.
            proj(0)
            proj(1)
            for ci in range(NCI):
                attn(ci, ao_tiles[ci])
                if ci + 2 < NCI:
                    proj(ci + 2)
                outproj(ci, ao_tiles[ci])

    nc.compile()
    return nc


def _round_f32r(a):
    """Round fp32 to the fp32r grid (11-bit mantissa, round-to-nearest)."""
    bits = np.ascontiguousarray(a, np.float32).view(np.uint32)
    return ((bits + 0x800) & 0xFFFFF000).view(np.float32)


def make_in_maps(x, freqs_cos, freqs_sin, wq, wk, wv, wo):
    """Host-side sharding + layout prep. Returns per-core input dicts."""
    x = np.asarray(x, np.float32)
    fc = np.asarray(freqs_cos, np.float32)
    fs = np.asarray(freqs_sin, np.float32)
    wq = np.asarray(wq, np.float32)
    wk = np.asarray(wk, np.float32)
    wv = np.asarray(wv, np.float32)
    wo = np.asarray(wo, np.float32)

    perm = np.concatenate([np.arange(0, HD, 2), np.arange(1, HD, 2)])
    cosT = np.ascontiguousarray(fc.T)            # (32, T)
    sinT = np.ascontiguousarray(fs.T)
    cosf = np.concatenate([cosT] * 4, axis=0)    # (128, T)
    sinf = np.concatenate([-sinT, sinT, -sinT, sinT], axis=0)

    jj = np.arange(128)[:, None]
    ii = np.arange(128)[None, :]
    msk = np.where(jj <= ii, 1.0, 0.0).astype(np.float32)
    ident = np.tile(np.eye(HD, dtype=np.float32), (2, 1))

    in_maps = []
    for c in range(N_CORES):
        b, g = divmod(c, 4)
        wq_c = wq[:, g * QCOLS:(g + 1) * QCOLS]
        wq_c = np.ascontiguousarray(
            wq_c.reshape(D, NQH, HD)[:, :, perm].reshape(D, QCOLS))
        wk_c = wk[:, g * HD:(g + 1) * HD][:, perm]
        wv_c = wv[:, g * HD:(g + 1) * HD]
        wkv_c = np.ascontiguousarray(np.concatenate([wk_c, wv_c], axis=1))
        wo_c = np.ascontiguousarray(wo[g * QCOLS:(g + 1) * QCOLS, :])
        xT_c = np.ascontiguousarray(x[b].T)
        in_maps.append({
            "xT": _round_f32r(xT_c), "wq": _round_f32r(wq_c),
            "wkv": _round_f32r(wkv_c), "wo": _round_f32r(wo_c),
            "cosf": cosf, "sinf": sinf, "msk": msk, "ident": ident,
            "ones1": np.ones((1, HD), np.float32),
        })
    return in_maps


def run_on_cores(in_maps, trace=False, **kwargs):
    if "nc" not in _cache:
        _cache["nc"] = build_nc()
    return run_bass_kernel_spmd(
        _cache["nc"], in_maps, core_ids=list(range(N_CORES)), trace=trace,
        **kwargs)


def kernel(x, freqs_cos, freqs_sin, wq, wk, wv, wo):
    in_maps = make_in_maps(x, freqs_cos, freqs_sin, wq, wk, wv, wo)
    res = run_on_cores(in_maps)
    outs = [res.results[c]["out"] for c in range(N_CORES)]
    full = np.empty((B, T, D), np.float32)
    for b in range(B):
        full[b] = outs[4 * b] + outs[4 * b + 1] + outs[4 * b + 2] + outs[4 * b + 3]
    return full
